# revision 1
# baseline (speedup 1.0000x reference)
"""Chunkwise SSM layer as a Bass/Tile kernel on 8 Trainium2 NeuronCores.

Math: the reference's inter-chunk correction cancels exactly
(h_next = Th + (h_final - Th) = h_final for ANY mix_weight), so the layer
reduces to a plain diagonal first-order scan:
    G  = sigmoid(x @ gate_W + gate_b)        (B,S,n)
    Bv = x @ B_W                             (B,S,n)
    h_t = G_t * h_{t-1} + Bv_t               (scan over S)
    out = (h @ C_W) * sigmoid(x @ out_W)     (B,S,d)

Sharding: (batch, seq-half) -> 8 cores. Second halves re-derive their
initial state with a W=512-token warmup scan (gate products decay ~e^-40
over 512 steps, so the truncation error is ~1e-17 relative) -- no
cross-core communication needed. First halves get a zero warmup (exact).

On-core layout: time stays on the free axis. x is transposed on the PE
(f32r transpose) into X^T [d, t] tiles which serve as rhs for the
gate/B projections (stacked into ONE accumulation: out partitions 0:64 =
G^T, 64:128 = Bv^T) and as stationary operand for the out-gate matmul in
natural [t, d] layout. The scan runs on the Vector engine via
tensor_tensor_scan (one recurrence per partition along the free axis),
chained across 512-token blocks through an initial-state AP.
All matmuls run in float32r (TF32-like, ~1.6e-4 rel err, 1 cycle/row).
"""

import numpy as np

_B, _S, _D, _N = 4, 4096, 1024, 64
_T = _S // 2  # main tokens per core
_W = 512      # warmup tokens (scan state re-derivation for second halves)
_TB = 512     # tokens per pipeline block
_NBLK = (_W + _T) // _TB  # 5 blocks: 1 warmup + 4 main

_cache = {}


def _build():
    import concourse.mybir as mybir
    import concourse.tile as tile
    from concourse import bacc
    from concourse.masks import make_identity

    F32, F32R = mybir.dt.float32, mybir.dt.float32r
    Sigmoid = mybir.ActivationFunctionType.Sigmoid
    MULT, ADD = mybir.AluOpType.mult, mybir.AluOpType.add

    nc = bacc.Bacc("TRN2", target_bir_lowering=False, debug=False, num_devices=8)

    xs = nc.dram_tensor("xs", [_W + _T, _D], F32R, kind="ExternalInput")
    wgb = nc.dram_tensor("wgb", [_D, 2 * _N], F32R, kind="ExternalInput")
    cw = nc.dram_tensor("cw", [_N, _D], F32R, kind="ExternalInput")
    ow = nc.dram_tensor("ow", [_D, _D], F32R, kind="ExternalInput")
    gbias = nc.dram_tensor("gbias", [_N, 1], F32, kind="ExternalInput")
    out = nc.dram_tensor("out", [_T, _D], F32, kind="ExternalOutput")

    KT = _D // 128  # 8 contraction tiles
    NT = _TB // 128  # 4 token tiles per block

    with tile.TileContext(nc) as tc:
        with (
            tc.tile_pool(name="singles", bufs=1) as singles,
            tc.tile_pool(name="xnat", bufs=2) as xnat_pool,
            tc.tile_pool(name="xtb", bufs=2) as xtb_pool,
            tc.tile_pool(name="gates", bufs=2) as gates_pool,
            tc.tile_pool(name="hpool", bufs=2) as h_pool,
            tc.tile_pool(name="opool", bufs=4) as o_pool,
            tc.tile_pool(name="tp_ps", bufs=2, space="PSUM") as tp_ps,
            tc.tile_pool(name="gb_ps", bufs=1, space="PSUM") as gb_ps,
            tc.tile_pool(name="og_ps", bufs=3, space="PSUM") as og_ps,
            tc.tile_pool(name="y_ps", bufs=1, space="PSUM") as y_ps,
        ):
            # ---- constants ----
            identf = singles.tile([128, 128], F32)
            make_identity(nc, identf[:])
            ident = singles.tile([128, 128], F32R)
            nc.vector.tensor_copy(ident[:], identf[:])

            gb_t = singles.tile([_N, 1], F32)
            nc.sync.dma_start(out=gb_t[:], in_=gbias.ap())

            wgb_t = singles.tile([128, KT, 2 * _N], F32R)
            nc.sync.dma_start(
                out=wgb_t[:], in_=wgb.ap().rearrange("(o p) m -> p o m", p=128)
            )
            cw_t = singles.tile([_N, _D], F32R)
            nc.sync.dma_start(out=cw_t[:], in_=cw.ap())
            ow_t = singles.tile([128, KT, _D], F32R)
            nc.sync.dma_start(
                out=ow_t[:], in_=ow.ap().rearrange("(o p) m -> p o m", p=128)
            )

            prev_ht = None
            for blk in range(_NBLK):
                r0 = blk * _TB
                # natural-layout x block [token-in-tile, tile, feature]
                xnat = xnat_pool.tile([128, NT, _D], F32R)
                nc.sync.dma_start(
                    out=xnat[:],
                    in_=xs.ap()[r0 : r0 + _TB, :].rearrange(
                        "(tt p) d -> p tt d", p=128
                    ),
                )
                # PE transpose -> X^T block [d-tile, k, token]
                xtb = xtb_pool.tile([128, KT, _TB], F32R)
                for dk in range(KT):
                    pt = tp_ps.tile([128, _TB], F32R, tag="tp")
                    for tt in range(NT):
                        nc.tensor.transpose(
                            pt[:, tt * 128 : (tt + 1) * 128],
                            xnat[:, tt, dk * 128 : (dk + 1) * 128],
                            ident[:],
                        )
                    if dk % 2 == 0:
                        nc.vector.tensor_copy(xtb[:, dk, :], pt[:])
                    else:
                        nc.scalar.copy(xtb[:, dk, :], pt[:])

                # gate/B projections: psum[0:64]=G^T logits, [64:128]=Bv^T
                gbp = gb_ps.tile([128, _TB], F32, tag="gb")
                for kk in range(KT):
                    nc.tensor.matmul(
                        gbp[:],
                        wgb_t[:, kk, :],
                        xtb[:, kk, :],
                        start=(kk == 0),
                        stop=(kk == KT - 1),
                    )
                st = gates_pool.tile([_N, _TB], F32, tag="st")
                nc.scalar.activation(
                    out=st[:], in_=gbp[:_N, :], func=Sigmoid, bias=gb_t[:], scale=1.0
                )
                bt = gates_pool.tile([_N, _TB], F32, tag="bt")
                nc.vector.tensor_copy(bt[:], gbp[_N:, :])

                # the scan: h = G*h + Bv along time, chained across blocks
                ht = h_pool.tile([_N, _TB], F32R)
                init = 0.0 if prev_ht is None else prev_ht[:, _TB - 1 : _TB]
                nc.vector.tensor_tensor_scan(
                    ht[:], st[:], bt[:], init, op0=MULT, op1=ADD
                )
                prev_ht = ht

                if blk == 0:
                    continue  # warmup block: only the state matters

                # out-gate + y + final product, natural [t, d] layout
                for tt in range(NT):
                    yp = y_ps.tile([128, _D], F32, tag="y")
                    for ck in range(2):
                        nc.tensor.matmul(
                            yp[:, ck * 512 : (ck + 1) * 512],
                            ht[:, tt * 128 : (tt + 1) * 128],
                            cw_t[:, ck * 512 : (ck + 1) * 512],
                            start=True,
                            stop=True,
                        )
                    ot = o_pool.tile([128, _D], F32, tag="ot")
                    for ck in range(2):
                        ogp = og_ps.tile([128, 512], F32, tag="og")
                        for kk in range(KT):
                            nc.tensor.matmul(
                                ogp[:],
                                xtb[:, kk, tt * 128 : (tt + 1) * 128],
                                ow_t[:, kk, ck * 512 : (ck + 1) * 512],
                                start=(kk == 0),
                                stop=(kk == KT - 1),
                            )
                        cs = slice(ck * 512, (ck + 1) * 512)
                        nc.scalar.activation(
                            out=ot[:, cs], in_=ogp[:], func=Sigmoid, bias=0.0, scale=1.0
                        )
                        nc.vector.tensor_mul(ot[:, cs], ot[:, cs], yp[:, cs])
                    nc.sync.dma_start(
                        out=out.ap()[
                            (blk - 1) * _TB + tt * 128 : (blk - 1) * _TB + (tt + 1) * 128,
                            :,
                        ],
                        in_=ot[:],
                    )
    nc.compile()
    return nc


def kernel(x, gate_W, gate_b, B_W, C_W, out_W, mix_weight, chunk_size):
    from concourse.bass_utils import run_bass_kernel_spmd

    x = np.ascontiguousarray(np.asarray(x), dtype=np.float32)
    assert x.shape == (_B, _S, _D), x.shape

    nc = _cache.get("nc")
    if nc is None:
        nc = _cache["nc"] = _build()

    wgb = np.ascontiguousarray(
        np.concatenate(
            [np.asarray(gate_W, np.float32), np.asarray(B_W, np.float32)], axis=1
        )
    )
    cw = np.ascontiguousarray(np.asarray(C_W, np.float32))
    ow = np.ascontiguousarray(np.asarray(out_W, np.float32))
    gbias = np.ascontiguousarray(np.asarray(gate_b, np.float32).reshape(_N, 1))

    zeros_warm = np.zeros((_W, _D), np.float32)
    in_maps = []
    for b in range(_B):
        for half in range(2):
            main = x[b, half * _T : (half + 1) * _T]
            warm = zeros_warm if half == 0 else x[b, _T - _W : _T]
            xs = np.ascontiguousarray(np.concatenate([warm, main], axis=0))
            in_maps.append(dict(xs=xs, wgb=wgb, cw=cw, ow=ow, gbias=gbias))

    res = run_bass_kernel_spmd(nc, in_maps, core_ids=list(range(8)))
    _cache["last_result"] = res

    out = np.empty((_B, _S, _D), np.float32)
    for i in range(8):
        b, half = divmod(i, 2)
        out[b, half * _T : (half + 1) * _T] = res.results[i]["out"]
    return out
